# revision 49
# baseline (speedup 1.0000x reference)
"""Trainium2 Bass kernel for nn_MultiHeadCrossAttention (B=4, T=1024, E=1024, H=16).

Sharding: 8 fully independent shards (output stream s, batch b), zero
cross-core communication. Core c<4 computes stream-1 batch c; core c>=4
stream-2 batch c-4. Stream-1 output needs K,V from x (Wqkv1) and Q from y
(Wqkv2); stream-2 the reverse.

Per-core structure (all activations fp16 on-chip, feature-on-partition):
  V     = A^T.T @ Wv^T   (natural [j, dv] layout + ones column per head;
                          e-outer accumulation overlaps the input DMA, with
                          the m=0 Q^T/K^T chains interleaved at the end)
  per head-pair m, query-half ic (units of 512 queries):
    Q^T/K^T chunks for m+1 via 8-matmul chains, 2 MMs interleaved per jc
    S^T[j,i] pair = K^T.T @ Q^T  (two heads concurrent via PE row split)
    P = exp(S/8) on ACT (single [128,1024] ACTIVATE per j-chunk, both heads)
    O'^T[d,i] += V.T @ P^T  (M=65 ones column -> rowsum at psum row 64;
                             O-pair lags one jc so it never waits on exp)
    norm: rowsums -> part-0 copies -> reciprocal_approx_fast (DVE) ->
          GpSimd partition_broadcast -> DVE mul -> ot fp16
    (m=7 units instead interleave the first two out-proj chains, cc=0,1)
  Z^T = Wout^T.T @ O^T for cc=2..7, fp16 out-DMA; host re-transposes/casts.

Measured per core: ~257us total; PE ~234us active (89%), ACT(exp) ~147us,
DVE ~45%. The attention loop is exp-throughput-paced; PSUM (8 banks) caps
deeper pipelining: S 2x[128,1024](4) + O 2x[65,512](2) + chains 2x[128,512].
"""

import os
import sys

sys.path.insert(0, "/opt/trn_rl_repo")

import numpy as np
from contextlib import ExitStack

import concourse.bass as bass
import concourse.mybir as mybir
import concourse.tile as tile
from concourse import bacc
from concourse import bass_utils

B, T, E, H = 4, 1024, 1024, 16
D = E // H            # 64
NC = E // 128         # 8 chunks of 128
N_CORES = 8

F32 = mybir.dt.float32
F16 = mybir.dt.float16

_NC_CACHE = {}
LAST_RESULTS = {}

KDBG = os.environ.get("KDBG", "")


def _build():
    nc = bacc.Bacc("TRN2", target_bir_lowering=False, debug=False,
                   enable_asserts=False, num_devices=N_CORES)
    a_t = nc.dram_tensor("a_t", (E, T), F16, kind="ExternalInput").ap()
    b_t = nc.dram_tensor("b_t", (E, T), F16, kind="ExternalInput").ap()
    wq_t = nc.dram_tensor("wq_t", (E, E), F16, kind="ExternalInput").ap()
    wk_t = nc.dram_tensor("wk_t", (E, E), F16, kind="ExternalInput").ap()
    wv_t = nc.dram_tensor("wv_t", (E, E), F16, kind="ExternalInput").ap()
    wout_t = nc.dram_tensor("wout_t", (E, E), F16, kind="ExternalInput").ap()
    z_t = nc.dram_tensor("z_t", (E, T), F16, kind="ExternalOutput").ap()

    with tile.TileContext(nc) as tc, ExitStack() as ctx:
        # ---------------- long-lived SBUF ----------------
        # inputs live as per-chunk tiles so DMA->compute deps are precise
        # (compute on chunk c starts as soon as chunk c lands)
        big = ctx.enter_context(tc.tile_pool(name="big", bufs=1))
        at_sb = [big.tile([128, T], F16, tag=f"at{c}", name=f"at{c}")
                 for c in range(NC)]
        bt_sb = [big.tile([128, T], F16, tag=f"bt{c}", name=f"bt{c}")
                 for c in range(NC)]
        wq_sb = [big.tile([128, E], F16, tag=f"wq{c}", name=f"wq{c}")
                 for c in range(NC)]
        wk_sb = [big.tile([128, E], F16, tag=f"wk{c}", name=f"wk{c}")
                 for c in range(NC)]
        wv_sb = [big.tile([128, E], F16, tag=f"wv{c}", name=f"wv{c}")
                 for c in range(NC)]
        wo_sb = [big.tile([128, E], F16, tag=f"wo{c}", name=f"wo{c}")
                 for c in range(NC)]
        v = big.tile([128, NC, H * (D + 1)], F16, tag="v")
        ot = big.tile([128, NC, T], F16, tag="ot")
        # rowsums are shift-copied to partition 0 (custom DVE recip only
        # supports base partition 0), reciprocal'd, then partition-broadcast
        # by GpSimd into the [128, 512] multiplier tile.
        rsfA = big.tile([1, 512], F32, tag="rsfA")
        rsfB = big.tile([1, 512], F32, tag="rsfB")
        rsgA = big.tile([1, 512], F32, tag="rsgA")
        rsgB = big.tile([1, 512], F32, tag="rsgB")

        # constants: ones columns in v; head-select matrix for the recip
        # broadcast matmul (recipA on partition 0 -> out rows 0-63, recipB on
        # partition 32 -> out rows 64-127; other partitions stay zero)
        for jc in range(NC):
            nc.vector.memset(
                v[:, jc, :].rearrange("p (h x) -> p h x", x=D + 1)[:, :, D:D + 1], 1.0)


        # ---------------- input DMA (order matters) ----------------
        for c in range(NC):
            nc.sync.dma_start(at_sb[c][:], a_t[c * 128:(c + 1) * 128, :])
            nc.sync.dma_start(wv_sb[c][:], wv_t[c * 128:(c + 1) * 128, :])
        for c in range(NC):
            nc.sync.dma_start(bt_sb[c][:], b_t[c * 128:(c + 1) * 128, :])
            nc.sync.dma_start(wq_sb[c][:], wq_t[c * 128:(c + 1) * 128, :])
            nc.sync.dma_start(wk_sb[c][:], wk_t[c * 128:(c + 1) * 128, :])
        for c in range(NC):
            nc.sync.dma_start(wo_sb[c][:], wout_t[c * 128:(c + 1) * 128, :])

        qt_pool = ctx.enter_context(tc.tile_pool(name="qt", bufs=3))
        kt_pool = ctx.enter_context(tc.tile_pool(name="kt", bufs=3))
        qt_tiles, kt_tiles = {}, {}

        def make_chain(m, which, psum_pool, psum_tag):
            """Incremental 2x8-matmul Q^T/K^T projection chain for head
            pair m; step(n) emits n matmuls, auto-finishing each ic-half
            with a PSUM->SBUF fp16 copy."""
            if m >= NC:
                return lambda n: None
            w_sb = wq_sb if which == 'q' else wk_sb
            act = bt_sb if which == 'q' else at_sb
            pool = qt_pool if which == 'q' else kt_pool
            tiles = qt_tiles if which == 'q' else kt_tiles
            st = {"e": 0, "ic": 0, "ps": None}

            def step(n):
                for _ in range(n):
                    ic2, e = st["ic"], st["e"]
                    if ic2 >= 2:
                        return
                    if e == 0:
                        st["ps"] = psum_pool.tile([128, 512], F32, tag=psum_tag,
                                                  name=f"{which}ps{m}_{ic2}")
                    nc.tensor.matmul(
                        st["ps"], w_sb[e][:, bass.ts(m, 128)],
                        act[e][:, bass.ts(ic2, 512)],
                        start=(e == 0), stop=(e == NC - 1))
                    if e == NC - 1:
                        if ic2 == 0:
                            tiles[m] = pool.tile([128, T], F16, tag="c",
                                                 name=f"{which}t{m}")
                        with nc.allow_low_precision(reason="fp16 q/k"):
                            nc.vector.tensor_copy(
                                tiles[m][:, bass.ts(ic2, 512)], st["ps"])
                        st["e"], st["ic"] = 0, ic2 + 1
                    else:
                        st["e"] = e + 1
            return step

        # ---------------- V projection: v[j, dv] = sum_e at[e,j] wv[e,dv] ----
        # e-outer over jc-groups so accumulation starts as soon as the
        # first at/wv chunks land (V compute overlaps its own input DMA).
        # The m=0 Q^T/K^T chains are interleaved into later groups so the
        # attention loop starts immediately after V.
        with tc.tile_pool(name="vps", bufs=3, space="PSUM") as vps, \
             tc.tile_pool(name="pps0", bufs=2, space="PSUM") as pps0:
            pro_q = make_chain(0, 'q', pps0, "p0")
            pro_k = make_chain(0, 'k', pps0, "p0")
            for gi, grp in enumerate(((0, 1, 2), (3, 4, 5), (6, 7))):
                pss = {jc: vps.tile([128, T], F32, tag="vp", name=f"vp{jc}")
                       for jc in grp}
                for e in range(NC):
                    for jc in grp:
                        for ic in range(2):
                            nc.tensor.matmul(
                                pss[jc][:, bass.ts(ic, 512)],
                                at_sb[e][:, bass.ts(jc, 128)],
                                wv_sb[e][:, bass.ts(ic, 512)],
                                start=(e == 0), stop=(e == NC - 1))
                    if gi == 1:
                        pro_q(2)
                    elif gi == 2:
                        pro_k(2)
                for jc in grp:
                    # ACT is idle during the V phase; keep DVE free
                    nc.scalar.copy(
                        v[:, jc, :].rearrange("p (h x) -> p h x", x=D + 1)[:, :, 0:D],
                        pss[jc][:].rearrange("p (h x) -> p h x", x=D))
            pro_q(16)
            pro_k(16)

        # ---------------- fused attention loop ----------------
        with tc.tile_pool(name="sps", bufs=2, space="PSUM") as sps, \
             tc.tile_pool(name="ops", bufs=1, space="PSUM") as ops, \
             tc.tile_pool(name="qkps", bufs=2, space="PSUM") as qkps, \
             tc.tile_pool(name="ptp", bufs=6) as ptp, \
             tc.tile_pool(name="zc", bufs=4) as zcp, \
             tc.tile_pool(name="nrm", bufs=4) as nrm:

            for m in range(NC):
                qt_m, kt_m = qt_tiles[m], kt_tiles[m]
                hA, hB = 2 * m, 2 * m + 1
                if KDBG in ("qt", "kt"):
                    src = qt_m if KDBG == "qt" else kt_m
                    nc.vector.tensor_copy(ot[:, m, :], src[:])
                    make_chain(m + 1, 'q', qkps, "qk")(16)
                    make_chain(m + 1, 'k', qkps, "qk")(16)
                    continue
                for ic in range(2):
                    s_ic = bass.ts(ic, 512)
                    if m + 1 < NC:
                        chain = make_chain(m + 1, 'q' if ic == 0 else 'k',
                                           qkps, "qk")
                        zfin = None
                    else:
                        # last head pair: fill PE gaps with the first two
                        # out-projection chains (cc=0,1) for this ic-half
                        zst = []
                        for cc in range(2):
                            zst.append({"mm": 0, "ps": None, "cc": cc})

                        def chain(n, _zst=zst, _ic=ic):
                            for _ in range(n):
                                zs = min(_zst, key=lambda z: z["mm"])
                                mm = zs["mm"]
                                if mm >= NC - 1:
                                    return
                                if mm == 0:
                                    zs["ps"] = qkps.tile(
                                        [128, 512], F32, tag="qk",
                                        name=f"zc{zs['cc']}_{_ic}")
                                nc.tensor.matmul(
                                    zs["ps"],
                                    wo_sb[mm][:, bass.ts(zs["cc"], 128)],
                                    ot[:, mm, bass.ts(_ic, 512)],
                                    start=(mm == 0), stop=False)
                                zs["mm"] = mm + 1

                        def zfin(_zst=zst, _ic=ic):
                            chain(16)  # drain both chains to mm = NC-2
                            for zs in _zst:
                                nc.tensor.matmul(
                                    zs["ps"],
                                    wo_sb[NC - 1][:, bass.ts(zs["cc"], 128)],
                                    ot[:, NC - 1, bass.ts(_ic, 512)],
                                    start=False, stop=True)
                                zt = zcp.tile([128, 512], F16, tag="zct",
                                              name=f"zct{zs['cc']}_{_ic}")
                                nc.scalar.copy(zt[:], zs["ps"])
                                nc.sync.dma_start(
                                    z_t[zs["cc"] * 128:(zs["cc"] + 1) * 128,
                                        _ic * 512:(_ic + 1) * 512], zt[:])
                    ps_oA = ops.tile([65, 512], F32, tag="oA")
                    ps_oB = ops.tile([65, 512], F32, tag="oB")
                    # O-pair lags one jc behind S/exp so it never waits on the
                    # exp latency (exp(jc-1) finished a full period ago)
                    ptcs = {}
                    for jc in range(NC + 1):
                        if jc < NC:
                            ps_s = sps.tile([128, T], F32, tag="s")
                            nc.tensor.matmul(
                                ps_s[:, 0:512],
                                kt_m[0:64, bass.ts(jc, 128)], qt_m[0:64, s_ic],
                                start=True, stop=True)
                            nc.tensor.matmul(
                                ps_s[:, 512:1024],
                                kt_m[64:128, bass.ts(jc, 128)], qt_m[64:128, s_ic],
                                start=True, stop=True)
                            ptc = ptp.tile([128, T], F16, tag="pt")
                            nc.scalar.activation(ptc[:], ps_s[:],
                                                 mybir.ActivationFunctionType.Exp,
                                                 scale=0.125)
                            ptcs[jc] = ptc
                            if KDBG == "pt" and jc == 0:
                                nc.vector.tensor_copy(ot[:, m, s_ic], ptc[:, 0:512])
                            chain(2)
                        if jc >= 1:
                            jo = jc - 1
                            ptco = ptcs.pop(jo)
                            st = dict(start=(jo == 0), stop=(jo == NC - 1))
                            nc.tensor.matmul(ps_oA, v[:, jo, bass.ts(hA, D + 1)],
                                             ptco[:, 0:512], **st)
                            nc.tensor.matmul(ps_oB, v[:, jo, bass.ts(hB, D + 1)],
                                             ptco[:, 512:1024], **st)

                    if KDBG == "pt":
                        continue
                    # rowsums + early PSUM evacuation (frees ps_oA/ps_oB for
                    # the next unit's accumulation before the norm completes)
                    oc = nrm.tile([128, 512], F16, tag="oc")
                    nc.vector.tensor_copy(rsfA[:], ps_oA[64:65, :])
                    nc.vector.tensor_copy(rsfB[:], ps_oB[64:65, :])
                    with nc.allow_low_precision(reason="fp16 O'"):
                        nc.vector.tensor_copy(oc[0:64, :], ps_oA[0:64, :])
                        nc.vector.tensor_copy(oc[64:128, :], ps_oB[0:64, :])
                    if KDBG == "onorm":
                        nc.vector.tensor_copy(ot[:, m, s_ic], oc[:])
                        continue
                    nc.vector.reciprocal_approx_fast(rsgA[:], rsfA[:])
                    nc.vector.reciprocal_approx_fast(rsgB[:], rsfB[:])
                    bcsA = nrm.tile([128, 512], F32, tag="bcsA")
                    bcsB = nrm.tile([128, 512], F32, tag="bcsB")
                    nc.gpsimd.partition_broadcast(bcsA[:], rsgA[:], channels=128)
                    nc.gpsimd.partition_broadcast(bcsB[:], rsgB[:], channels=128)
                    with nc.allow_low_precision(reason="fp16 O out"):
                        nc.vector.tensor_mul(ot[0:64, m, s_ic], oc[0:64, :],
                                             bcsA[0:64, :])
                        nc.vector.tensor_mul(ot[64:128, m, s_ic], oc[64:128, :],
                                             bcsB[64:128, :])
                        if KDBG == "bcs":
                            nc.vector.tensor_copy(ot[0:64, m, s_ic], bcsA[0:64, :])
                            nc.vector.tensor_copy(ot[64:128, m, s_ic],
                                                  bcsB[64:128, :])
                    if zfin is not None:
                        zfin()


        # ---------------- out-projection ----------------
        if KDBG:
            for cc in range(NC):
                nc.sync.dma_start(z_t[cc * 128:(cc + 1) * 128, :], ot[:, cc, :])
        else:
            with tc.tile_pool(name="zps", bufs=2, space="PSUM") as zps, \
                 tc.tile_pool(name="zsb", bufs=2) as zsbp:
                for cc in range(2, NC):
                    ps = zps.tile([128, T], F32, tag="z")
                    for mm in range(NC):
                        for ic in range(2):
                            nc.tensor.matmul(
                                ps[:, bass.ts(ic, 512)],
                                wo_sb[mm][:, bass.ts(cc, 128)],
                                ot[:, mm, bass.ts(ic, 512)],
                                start=(mm == 0), stop=(mm == NC - 1))
                    zsb = zsbp.tile([128, T], F16, tag="zsb")
                    nc.scalar.copy(zsb[:], ps[:])
                    nc.sync.dma_start(z_t[cc * 128:(cc + 1) * 128, :], zsb[:])
    nc.compile()
    return nc


def _group_w(wqkv, k):
    """Rows of Wqkv (3E, E) for q/k/v (k=0/1/2), grouped head-major.

    Row index layout: r = di*(3H) + k*H + h  ->  grouped[h*D+di, :].
    """
    w = np.asarray(wqkv, dtype=np.float32).reshape(D, 3, H, E)[:, k]   # [di, h, e]
    return np.ascontiguousarray(w.transpose(1, 0, 2).reshape(E, E))    # [h*D+di, e]


def kernel(x, y, Wqkv1, Wqkv2, Wout1, Wout2):
    x = np.asarray(x, dtype=np.float32)
    y = np.asarray(y, dtype=np.float32)

    if "nc" not in _NC_CACHE:
        _NC_CACHE["nc"] = _build()
    nc = _NC_CACHE["nc"]

    wq1_t = np.ascontiguousarray(_group_w(Wqkv1, 0).T)
    wk1_t = np.ascontiguousarray(_group_w(Wqkv1, 1).T)
    wv1_t = np.ascontiguousarray(_group_w(Wqkv1, 2).T)
    wq2_t = np.ascontiguousarray(_group_w(Wqkv2, 0).T)
    wk2_t = np.ascontiguousarray(_group_w(Wqkv2, 1).T)
    wv2_t = np.ascontiguousarray(_group_w(Wqkv2, 2).T)
    wout1_t = np.ascontiguousarray(np.asarray(Wout1, dtype=np.float32).T)
    wout2_t = np.ascontiguousarray(np.asarray(Wout2, dtype=np.float32).T)

    in_maps = []
    for c in range(N_CORES):
        s, b = divmod(c, B)
        if s == 0:
            # stream-1 output: K,V from x via Wqkv1; Q from y via Wqkv2
            a_t, b_t = x[b].T, y[b].T
            wq, wk, wv, wo = wq2_t, wk1_t, wv1_t, wout1_t
        else:
            a_t, b_t = y[b].T, x[b].T
            wq, wk, wv, wo = wq1_t, wk2_t, wv2_t, wout2_t
        in_maps.append({
            "a_t": np.ascontiguousarray(a_t).astype(np.float16),
            "b_t": np.ascontiguousarray(b_t).astype(np.float16),
            "wq_t": wq.astype(np.float16), "wk_t": wk.astype(np.float16),
            "wv_t": wv.astype(np.float16), "wout_t": wo.astype(np.float16),
        })

    trace = os.environ.get("BASS_KERNEL_TRACE", "0") == "1"
    if trace:
        try:
            from antenv.axon_hooks import get_axon_ntff_profile_hook  # noqa: F401
        except ImportError:
            trace = False
    ncores = int(os.environ.get("KCORES", str(N_CORES)))
    r = bass_utils.run_bass_kernel_spmd(nc, in_maps[:ncores], core_ids=list(range(ncores)),
                                        trace=trace)
    LAST_RESULTS["exec_time_ns"] = r.exec_time_ns
    LAST_RESULTS["profile_json"] = r.profile_json

    out1 = np.stack([r.results[b]["z_t"].T for b in range(B)]).astype(np.float32)
    out2 = np.stack([r.results[B + b]["z_t"].T for b in range(B)]).astype(np.float32)
    return out1, out2


# revision 50
# speedup vs baseline: 1.0494x; 1.0494x over previous
"""Trainium2 Bass kernel for nn_MultiHeadCrossAttention (B=4, T=1024, E=1024, H=16).

Sharding: 8 fully independent shards (output stream s, batch b), zero
cross-core communication. Core c<4 computes stream-1 batch c; core c>=4
stream-2 batch c-4. Stream-1 output needs K,V from x (Wqkv1) and Q from y
(Wqkv2); stream-2 the reverse.

Per-core structure (all activations fp16 on-chip, feature-on-partition):
  V     = A^T.T @ Wv^T   (natural [j, dv] layout + ones column per head;
                          e-outer accumulation overlaps the input DMA, with
                          the m=0 Q^T/K^T chains interleaved at the end)
  per head-pair m, query-half ic (units of 512 queries):
    Q^T/K^T chunks for m+1 via 8-matmul chains, 2 MMs interleaved per jc
    S^T[j,i] pair = K^T.T @ Q^T  (two heads concurrent via PE row split)
    P = exp(S/8) on ACT (single [128,1024] ACTIVATE per j-chunk, both heads)
    O'^T[d,i] += V.T @ P^T  (M=65 ones column -> rowsum at psum row 64;
                             O-pair lags one jc so it never waits on exp)
    norm: rowsums -> part-0 copies -> reciprocal_approx_fast (DVE) ->
          GpSimd partition_broadcast -> DVE mul -> ot fp16
    (m=7 units instead interleave the first two out-proj chains, cc=0,1)
  Z^T = Wout^T.T @ O^T for cc=2..7, fp16 out-DMA; host re-transposes/casts.

Measured per core: ~257us total; PE ~234us active (89%), ACT(exp) ~147us,
DVE ~45%. The attention loop is exp-throughput-paced; PSUM (8 banks) caps
deeper pipelining: S 2x[128,1024](4) + O 2x[65,512](2) + chains 2x[128,512].
"""

import os
import sys

sys.path.insert(0, "/opt/trn_rl_repo")

import numpy as np
from contextlib import ExitStack

import concourse.bass as bass
import concourse.mybir as mybir
import concourse.tile as tile
from concourse import bacc
from concourse import bass_utils

B, T, E, H = 4, 1024, 1024, 16
D = E // H            # 64
NC = E // 128         # 8 chunks of 128
N_CORES = 8

F32 = mybir.dt.float32
F16 = mybir.dt.float16

_NC_CACHE = {}
LAST_RESULTS = {}

KDBG = os.environ.get("KDBG", "")


def _build():
    nc = bacc.Bacc("TRN2", target_bir_lowering=False, debug=False,
                   enable_asserts=False, num_devices=N_CORES)
    a_t = nc.dram_tensor("a_t", (E, T), F16, kind="ExternalInput").ap()
    b_t = nc.dram_tensor("b_t", (E, T), F16, kind="ExternalInput").ap()
    wq_t = nc.dram_tensor("wq_t", (E, E), F16, kind="ExternalInput").ap()
    wk_t = nc.dram_tensor("wk_t", (E, E), F16, kind="ExternalInput").ap()
    wv_t = nc.dram_tensor("wv_t", (E, E), F16, kind="ExternalInput").ap()
    wout_t = nc.dram_tensor("wout_t", (E, E), F16, kind="ExternalInput").ap()
    z_t = nc.dram_tensor("z_t", (E, T), F16, kind="ExternalOutput").ap()

    with tile.TileContext(nc) as tc, ExitStack() as ctx:
        # ---------------- long-lived SBUF ----------------
        # inputs live as per-chunk tiles so DMA->compute deps are precise
        # (compute on chunk c starts as soon as chunk c lands)
        big = ctx.enter_context(tc.tile_pool(name="big", bufs=1))
        at_sb = [big.tile([128, T], F16, tag=f"at{c}", name=f"at{c}")
                 for c in range(NC)]
        bt_sb = [big.tile([128, T], F16, tag=f"bt{c}", name=f"bt{c}")
                 for c in range(NC)]
        wq_sb = [big.tile([128, E], F16, tag=f"wq{c}", name=f"wq{c}")
                 for c in range(NC)]
        wk_sb = [big.tile([128, E], F16, tag=f"wk{c}", name=f"wk{c}")
                 for c in range(NC)]
        wv_sb = [big.tile([128, E], F16, tag=f"wv{c}", name=f"wv{c}")
                 for c in range(NC)]
        wo_sb = [big.tile([128, E], F16, tag=f"wo{c}", name=f"wo{c}")
                 for c in range(NC)]
        v = big.tile([128, NC, H * (D + 1)], F16, tag="v")
        ot = big.tile([128, NC, T], F16, tag="ot")
        # rowsums are shift-copied to partition 0 (custom DVE recip only
        # supports base partition 0), reciprocal'd, then partition-broadcast
        # by GpSimd into the [128, 512] multiplier tile.
        rsfA = big.tile([1, 512], F32, tag="rsfA")
        rsfB = big.tile([1, 512], F32, tag="rsfB")
        rsgA = big.tile([1, 512], F32, tag="rsgA")
        rsgB = big.tile([1, 512], F32, tag="rsgB")

        # constants: ones columns in v; head-select matrix for the recip
        # broadcast matmul (recipA on partition 0 -> out rows 0-63, recipB on
        # partition 32 -> out rows 64-127; other partitions stay zero)
        for jc in range(NC):
            nc.vector.memset(
                v[:, jc, :].rearrange("p (h x) -> p h x", x=D + 1)[:, :, D:D + 1], 1.0)


        # ---------------- input DMA (order matters) ----------------
        for c in range(NC):
            nc.sync.dma_start(at_sb[c][:], a_t[c * 128:(c + 1) * 128, :])
            nc.sync.dma_start(wv_sb[c][:], wv_t[c * 128:(c + 1) * 128, :])
        for c in range(NC):
            nc.sync.dma_start(bt_sb[c][:], b_t[c * 128:(c + 1) * 128, :])
            nc.sync.dma_start(wq_sb[c][:], wq_t[c * 128:(c + 1) * 128, :])
            nc.sync.dma_start(wk_sb[c][:], wk_t[c * 128:(c + 1) * 128, :])
        for c in range(NC):
            nc.sync.dma_start(wo_sb[c][:], wout_t[c * 128:(c + 1) * 128, :])

        qt_pool = ctx.enter_context(tc.tile_pool(name="qt", bufs=3))
        kt_pool = ctx.enter_context(tc.tile_pool(name="kt", bufs=3))
        qt_tiles, kt_tiles = {}, {}

        def make_chain(m, which, psum_pool, psum_tag):
            """Incremental 2x8-matmul Q^T/K^T projection chain for head
            pair m; step(n) emits n matmuls, auto-finishing each ic-half
            with a PSUM->SBUF fp16 copy."""
            if m >= NC:
                return lambda n: None
            w_sb = wq_sb if which == 'q' else wk_sb
            act = bt_sb if which == 'q' else at_sb
            pool = qt_pool if which == 'q' else kt_pool
            tiles = qt_tiles if which == 'q' else kt_tiles
            st = {"e": 0, "ic": 0, "ps": None}

            def step(n):
                for _ in range(n):
                    ic2, e = st["ic"], st["e"]
                    if ic2 >= 2:
                        return
                    if e == 0:
                        st["ps"] = psum_pool.tile([128, 512], F32, tag=psum_tag,
                                                  name=f"{which}ps{m}_{ic2}")
                    nc.tensor.matmul(
                        st["ps"], w_sb[e][:, bass.ts(m, 128)],
                        act[e][:, bass.ts(ic2, 512)],
                        start=(e == 0), stop=(e == NC - 1))
                    if e == NC - 1:
                        if ic2 == 0:
                            tiles[m] = pool.tile([128, T], F16, tag="c",
                                                 name=f"{which}t{m}")
                        with nc.allow_low_precision(reason="fp16 q/k"):
                            nc.vector.tensor_copy(
                                tiles[m][:, bass.ts(ic2, 512)], st["ps"])
                        st["e"], st["ic"] = 0, ic2 + 1
                    else:
                        st["e"] = e + 1
            return step

        # ---------------- V projection: v[j, dv] = sum_e at[e,j] wv[e,dv] ----
        # e-outer over jc-groups so accumulation starts as soon as the
        # first at/wv chunks land (V compute overlaps its own input DMA).
        # The m=0 Q^T/K^T chains are interleaved into later groups so the
        # attention loop starts immediately after V.
        with tc.tile_pool(name="vps", bufs=3, space="PSUM") as vps, \
             tc.tile_pool(name="pps0", bufs=2, space="PSUM") as pps0:
            pro_q = make_chain(0, 'q', pps0, "p0")
            pro_k = make_chain(0, 'k', pps0, "p0")
            for gi, grp in enumerate(((0, 1, 2), (3, 4, 5), (6, 7))):
                pss = {jc: vps.tile([128, T], F32, tag="vp", name=f"vp{jc}")
                       for jc in grp}
                for e in range(NC):
                    for jc in grp:
                        for ic in range(2):
                            nc.tensor.matmul(
                                pss[jc][:, bass.ts(ic, 512)],
                                at_sb[e][:, bass.ts(jc, 128)],
                                wv_sb[e][:, bass.ts(ic, 512)],
                                start=(e == 0), stop=(e == NC - 1))
                    if gi == 1:
                        pro_q(2)
                    elif gi == 2:
                        pro_k(2)
                for jc in grp:
                    # ACT is idle during the V phase; keep DVE free
                    nc.scalar.copy(
                        v[:, jc, :].rearrange("p (h x) -> p h x", x=D + 1)[:, :, 0:D],
                        pss[jc][:].rearrange("p (h x) -> p h x", x=D))
            pro_q(16)
            pro_k(16)

        # ---------------- fused attention loop ----------------
        with tc.tile_pool(name="sps", bufs=2, space="PSUM") as sps, \
             tc.tile_pool(name="ops", bufs=1, space="PSUM") as ops, \
             tc.tile_pool(name="qkps", bufs=2, space="PSUM") as qkps, \
             tc.tile_pool(name="ptp", bufs=6) as ptp, \
             tc.tile_pool(name="zc", bufs=4) as zcp, \
             tc.tile_pool(name="nrm", bufs=4) as nrm:

            if KDBG in ("qt", "kt"):
                for m in range(NC):
                    src = qt_tiles[m] if KDBG == "qt" else kt_tiles[m]
                    nc.vector.tensor_copy(ot[:, m, :], src[:])
                    make_chain(m + 1, 'q', qkps, "qk")(16)
                    make_chain(m + 1, 'k', qkps, "qk")(16)
                units = []
            else:
                units = [(m, ic) for m in range(NC) for ic in range(2)]

            # one flattened pipeline over all (unit, jc) chunks: the O stream
            # lags the S/exp stream by LAG chunks, crossing unit boundaries so
            # ACT never drains while PE runs an O epilogue + norm
            LAG = 2
            chains, zsts, psos, ptcs = {}, {}, {}, {}

            def make_zchain(ic):
                zst = [{"mm": 0, "ps": None, "cc": cc} for cc in range(2)]

                def step(n):
                    for _ in range(n):
                        zs = min(zst, key=lambda z: z["mm"])
                        mm = zs["mm"]
                        if mm >= NC - 1:
                            return
                        if mm == 0:
                            zs["ps"] = qkps.tile([128, 512], F32, tag="qk",
                                                 name=f"zc{zs['cc']}_{ic}")
                        nc.tensor.matmul(
                            zs["ps"], wo_sb[mm][:, bass.ts(zs["cc"], 128)],
                            ot[:, mm, bass.ts(ic, 512)],
                            start=(mm == 0), stop=False)
                        zs["mm"] = mm + 1
                return step, zst

            def norm(u):
                m, ic = units[u]
                s_ic = bass.ts(ic, 512)
                ps_oA, ps_oB = psos.pop(u)
                oc = nrm.tile([128, 512], F16, tag="oc", name=f"oc{u}")
                nc.vector.tensor_copy(rsfA[:], ps_oA[64:65, :])
                nc.vector.tensor_copy(rsfB[:], ps_oB[64:65, :])
                with nc.allow_low_precision(reason="fp16 O'"):
                    nc.vector.tensor_copy(oc[0:64, :], ps_oA[0:64, :])
                    nc.vector.tensor_copy(oc[64:128, :], ps_oB[0:64, :])
                if KDBG == "onorm":
                    nc.vector.tensor_copy(ot[:, m, s_ic], oc[:])
                    return
                nc.vector.reciprocal_approx_fast(rsgA[:], rsfA[:])
                nc.vector.reciprocal_approx_fast(rsgB[:], rsfB[:])
                bcsA = nrm.tile([128, 512], F32, tag="bcsA", name=f"bA{u}")
                bcsB = nrm.tile([128, 512], F32, tag="bcsB", name=f"bB{u}")
                nc.gpsimd.partition_broadcast(bcsA[:], rsgA[:], channels=128)
                nc.gpsimd.partition_broadcast(bcsB[:], rsgB[:], channels=128)
                with nc.allow_low_precision(reason="fp16 O out"):
                    nc.vector.tensor_mul(ot[0:64, m, s_ic], oc[0:64, :],
                                         bcsA[0:64, :])
                    nc.vector.tensor_mul(ot[64:128, m, s_ic], oc[64:128, :],
                                         bcsB[64:128, :])
                    if KDBG == "bcs":
                        nc.vector.tensor_copy(ot[0:64, m, s_ic], bcsA[0:64, :])
                        nc.vector.tensor_copy(ot[64:128, m, s_ic],
                                              bcsB[64:128, :])
                if m == NC - 1:
                    # finish the pulled-in out-projection chains (cc=0,1)
                    chains[u](16)
                    for zs in zsts[u]:
                        nc.tensor.matmul(
                            zs["ps"], wo_sb[NC - 1][:, bass.ts(zs["cc"], 128)],
                            ot[:, NC - 1, bass.ts(ic, 512)],
                            start=False, stop=True)
                        zt = zcp.tile([128, 512], F16, tag="zct",
                                      name=f"zct{zs['cc']}_{ic}")
                        nc.scalar.copy(zt[:], zs["ps"])
                        nc.sync.dma_start(
                            z_t[zs["cc"] * 128:(zs["cc"] + 1) * 128,
                                ic * 512:(ic + 1) * 512], zt[:])

            for t in range(len(units) * NC + LAG):
                su, sjc = divmod(t, NC)
                if su < len(units):
                    m, ic = units[su]
                    s_ic = bass.ts(ic, 512)
                    qt_m, kt_m = qt_tiles[m], kt_tiles[m]
                    if sjc == 0:
                        if m + 1 < NC:
                            chains[su] = make_chain(
                                m + 1, 'q' if ic == 0 else 'k', qkps, "qk")
                        else:
                            chains[su], zsts[su] = make_zchain(ic)
                    ps_s = sps.tile([128, T], F32, tag="s", name=f"s{t}")
                    nc.tensor.matmul(
                        ps_s[:, 0:512],
                        kt_m[0:64, bass.ts(sjc, 128)], qt_m[0:64, s_ic],
                        start=True, stop=True)
                    nc.tensor.matmul(
                        ps_s[:, 512:1024],
                        kt_m[64:128, bass.ts(sjc, 128)], qt_m[64:128, s_ic],
                        start=True, stop=True)
                    ptc = ptp.tile([128, T], F16, tag="pt", name=f"pt{t}")
                    nc.scalar.activation(ptc[:], ps_s[:],
                                         mybir.ActivationFunctionType.Exp,
                                         scale=0.125)
                    ptcs[t] = ptc
                    if KDBG == "pt" and sjc == 0:
                        nc.vector.tensor_copy(ot[:, m, s_ic], ptc[:, 0:512])
                    chains[su](2)
                to = t - LAG
                ou, ojc = divmod(to, NC)
                if 0 <= ou < len(units):
                    m, ic = units[ou]
                    hA, hB = 2 * m, 2 * m + 1
                    if ojc == 0:
                        psos[ou] = (
                            ops.tile([65, 512], F32, tag="oA", name=f"oA{ou}"),
                            ops.tile([65, 512], F32, tag="oB", name=f"oB{ou}"))
                    ps_oA, ps_oB = psos[ou]
                    ptco = ptcs.pop(to)
                    st = dict(start=(ojc == 0), stop=(ojc == NC - 1))
                    nc.tensor.matmul(ps_oA, v[:, ojc, bass.ts(hA, D + 1)],
                                     ptco[:, 0:512], **st)
                    nc.tensor.matmul(ps_oB, v[:, ojc, bass.ts(hB, D + 1)],
                                     ptco[:, 512:1024], **st)
                    if ojc == NC - 1 and KDBG != "pt":
                        norm(ou)


        # ---------------- out-projection ----------------
        if KDBG:
            for cc in range(NC):
                nc.sync.dma_start(z_t[cc * 128:(cc + 1) * 128, :], ot[:, cc, :])
        else:
            with tc.tile_pool(name="zps", bufs=2, space="PSUM") as zps, \
                 tc.tile_pool(name="zsb", bufs=2) as zsbp:
                for cc in range(2, NC):
                    ps = zps.tile([128, T], F32, tag="z")
                    for mm in range(NC):
                        for ic in range(2):
                            nc.tensor.matmul(
                                ps[:, bass.ts(ic, 512)],
                                wo_sb[mm][:, bass.ts(cc, 128)],
                                ot[:, mm, bass.ts(ic, 512)],
                                start=(mm == 0), stop=(mm == NC - 1))
                    zsb = zsbp.tile([128, T], F16, tag="zsb")
                    nc.scalar.copy(zsb[:], ps[:])
                    nc.sync.dma_start(z_t[cc * 128:(cc + 1) * 128, :], zsb[:])
    nc.compile()
    return nc


def _group_w(wqkv, k):
    """Rows of Wqkv (3E, E) for q/k/v (k=0/1/2), grouped head-major.

    Row index layout: r = di*(3H) + k*H + h  ->  grouped[h*D+di, :].
    """
    w = np.asarray(wqkv, dtype=np.float32).reshape(D, 3, H, E)[:, k]   # [di, h, e]
    return np.ascontiguousarray(w.transpose(1, 0, 2).reshape(E, E))    # [h*D+di, e]


def kernel(x, y, Wqkv1, Wqkv2, Wout1, Wout2):
    x = np.asarray(x, dtype=np.float32)
    y = np.asarray(y, dtype=np.float32)

    if "nc" not in _NC_CACHE:
        _NC_CACHE["nc"] = _build()
    nc = _NC_CACHE["nc"]

    wq1_t = np.ascontiguousarray(_group_w(Wqkv1, 0).T)
    wk1_t = np.ascontiguousarray(_group_w(Wqkv1, 1).T)
    wv1_t = np.ascontiguousarray(_group_w(Wqkv1, 2).T)
    wq2_t = np.ascontiguousarray(_group_w(Wqkv2, 0).T)
    wk2_t = np.ascontiguousarray(_group_w(Wqkv2, 1).T)
    wv2_t = np.ascontiguousarray(_group_w(Wqkv2, 2).T)
    wout1_t = np.ascontiguousarray(np.asarray(Wout1, dtype=np.float32).T)
    wout2_t = np.ascontiguousarray(np.asarray(Wout2, dtype=np.float32).T)

    in_maps = []
    for c in range(N_CORES):
        s, b = divmod(c, B)
        if s == 0:
            # stream-1 output: K,V from x via Wqkv1; Q from y via Wqkv2
            a_t, b_t = x[b].T, y[b].T
            wq, wk, wv, wo = wq2_t, wk1_t, wv1_t, wout1_t
        else:
            a_t, b_t = y[b].T, x[b].T
            wq, wk, wv, wo = wq1_t, wk2_t, wv2_t, wout2_t
        in_maps.append({
            "a_t": np.ascontiguousarray(a_t).astype(np.float16),
            "b_t": np.ascontiguousarray(b_t).astype(np.float16),
            "wq_t": wq.astype(np.float16), "wk_t": wk.astype(np.float16),
            "wv_t": wv.astype(np.float16), "wout_t": wo.astype(np.float16),
        })

    trace = os.environ.get("BASS_KERNEL_TRACE", "0") == "1"
    if trace:
        try:
            from antenv.axon_hooks import get_axon_ntff_profile_hook  # noqa: F401
        except ImportError:
            trace = False
    ncores = int(os.environ.get("KCORES", str(N_CORES)))
    r = bass_utils.run_bass_kernel_spmd(nc, in_maps[:ncores], core_ids=list(range(ncores)),
                                        trace=trace)
    LAST_RESULTS["exec_time_ns"] = r.exec_time_ns
    LAST_RESULTS["profile_json"] = r.profile_json

    out1 = np.stack([r.results[b]["z_t"].T for b in range(B)]).astype(np.float32)
    out2 = np.stack([r.results[B + b]["z_t"].T for b in range(B)]).astype(np.float32)
    return out1, out2
